# revision 1
# baseline (speedup 1.0000x reference)
"""Trainium2 Bass kernel for nn_DownBlock_res_dct1 (maxpool 2x2 + truncated
block-DCT low-pass + SE attention + 1x1 conv + two 3x3 convs), data-parallel
over the batch across 8 NeuronCores.

Self-contained: hardcodes all shapes/constants; builds one SPMD Bass module
(one batch item per core), runs via run_bass_kernel_spmd, gathers the full
(8, 128, 192, 192) output.

Per-core layout: partitions p = half*64 + ch, where half selects image row
halves. All pre-conv compute runs in this layout; convs run per-half with
K=64 (conv1/att) or K=128 (conv2) contractions, N = 2 image rows per PSUM
tile. Matmuls in bf16 with fp32 PSUM accumulation.
"""

import math
from contextlib import ExitStack

import numpy as np

import concourse.bass as bass
import concourse.mybir as mybir
import concourse.tile as tile
from concourse import bacc
from concourse.bass_utils import run_bass_kernel_spmd

FP32 = mybir.dt.float32
BF16 = mybir.dt.bfloat16
AX = mybir.AxisListType
OP = mybir.AluOpType
ACT = mybir.ActivationFunctionType

N = 8  # DCT block size
_P8 = np.arange(8)
COS1 = np.cos(math.pi * (_P8 + 0.5) / 8.0 * 1).astype(np.float64)
COS2 = np.cos(math.pi * (_P8 + 0.5) / 8.0 * 2).astype(np.float64)
# Selected zigzag coeffs [0,1,2,5] -> (k1,k2) in {(0,0),(0,1),(1,0),(0,2)}
A00 = (1.0 / 8.0) ** 2
A01 = 2.0 / 64.0
A02 = 2.0 / 64.0
A10 = 2.0 / 64.0


def _runs(idx):
    """Contiguous runs where idx[i] = i - g: list of (out_start, in_start, len)."""
    runs = []
    s = 0
    for i in range(1, len(idx) + 1):
        if i == len(idx) or idx[i] != idx[i - 1] + 1:
            runs.append((s, int(idx[s]), i - s))
            s = i
    return runs


def _split_at(ro, rin, rl, bound):
    """Split a run at source-row `bound`."""
    if rin < bound < rin + rl:
        return [(ro, rin, bound - rin), (ro + bound - rin, bound, rin + rl - bound)]
    return [(ro, rin, rl)]


def build_nc(H=384, W=384, debug=False):
    C, C2 = 64, 128
    h, w = H // 2, W // 2
    hh = h // 2  # rows per half
    assert hh % N == 0 and w % N == 0
    T = hh // N  # block-rows per half
    S = w // N  # block-cols
    CH = 64

    hi = (np.arange(h) * (h - (N - 1))) // h
    wi = (np.arange(w) * (w - (N - 1))) // w
    col_runs = _runs(wi)
    row_runs_h = [_runs(hi[hh * hf : hh * (hf + 1)]) for hf in (0, 1)]

    MP_CH = 4  # pooled rows per maxpool chunk
    n_mp = hh // MP_CH
    DCT_T = 4 if T % 4 == 0 else (2 if T % 2 == 0 else T)
    n_dct = T // DCT_T
    XPC = DCT_T * N  # xp tile rows
    mp_per_xpc = XPC // MP_CH
    assert XPC % MP_CH == 0

    nc = bacc.Bacc("TRN2")

    x = nc.dram_tensor("x", [C, H, W], FP32, kind="ExternalInput")
    w1 = nc.dram_tensor("w1", [C2, C, 3, 3], FP32, kind="ExternalInput")
    b1 = nc.dram_tensor("b1", [C2], FP32, kind="ExternalInput")
    w2 = nc.dram_tensor("w2", [C2, C2, 3, 3], FP32, kind="ExternalInput")
    b2 = nc.dram_tensor("b2", [C2], FP32, kind="ExternalInput")
    attw = nc.dram_tensor("att_conv_w", [C, C, 1, 1], FP32, kind="ExternalInput")
    attb = nc.dram_tensor("att_conv_b", [C], FP32, kind="ExternalInput")
    fc1 = nc.dram_tensor("fc1_w", [C // 16, C, 1, 1], FP32, kind="ExternalInput")
    fc2 = nc.dram_tensor("fc2_w", [C, C // 16, 1, 1], FP32, kind="ExternalInput")
    out = nc.dram_tensor("out", [C2, h, w], FP32, kind="ExternalOutput")

    dbg = {}
    if debug:
        for name, shape in [
            ("dbg_xp", [128, hh, w]),
            ("dbg_y1", [128, hh, w]),
            ("dbg_se", [64, 8]),
            ("dbg_gamma", [64, 1]),
            ("dbg_xall", [128, hh + 2, w + 2]),
            ("dbg_o1", [C2, h + 2, w + 2]),
        ]:
            dbg[name] = nc.dram_tensor(name, shape, FP32, kind="ExternalOutput")

    const_np = np.zeros((128, 4, 8), np.float32)
    const_np[:, 0, :] = COS1
    const_np[:, 1, :] = COS2
    const_np[:, 2, :] = COS1 * A01
    const_np[:, 3, :] = COS2 * A02
    cdram = nc.inline_tensor(const_np.reshape(128, 32), name="dctconst")

    NF = h * w  # pixels per full channel image

    with tile.TileContext(nc) as tc, ExitStack() as ctx:
        wpool = ctx.enter_context(tc.tile_pool(name="wpool", bufs=1))
        if debug:
            dpool = ctx.enter_context(tc.tile_pool(name="dpool", bufs=1))
        smallD = ctx.enter_context(tc.tile_pool(name="smallD", bufs=1))
        small = ctx.enter_context(tc.tile_pool(name="small", bufs=2))
        psA = ctx.enter_context(tc.tile_pool(name="psA", bufs=2, space="PSUM"))
        psC = ctx.enter_context(tc.tile_pool(name="psC", bufs=4, space="PSUM"))
        # phase-scoped pools; each SBUF side is a LIFO stack.
        # left:  ... pxp prec | pop prec, pop pxp, push pxa, push po1
        # right: py1 pin      | pop pin (after maxpool), pop py1 (at end)
        py1 = tc.alloc_tile_pool(name="py1", bufs=1, side="right")
        pxp = tc.alloc_tile_pool(name="pxp", bufs=1, side="right")
        pin = tc.alloc_tile_pool(name="pin", bufs=2, side="right")
        prec = tc.alloc_tile_pool(name="prec", bufs=1)

        # ---------------- constants / weights ----------------
        consts = wpool.tile([128, 4, 8], FP32)
        nc.sync.dma_start(consts[:], cdram[:].rearrange("p (a b) -> p a b", a=4))

        def cvec(row, shp):  # broadcast [128,8] const row to shp (q innermost)
            return consts[:, row, None, None, :].to_broadcast(shp)

        from concourse.masks import make_identity

        ident = wpool.tile([128, 128], FP32)
        make_identity(nc, ident[:])

        zerot = wpool.tile([128, 1], FP32)
        nc.vector.memset(zerot[:], 0.0)

        w1s = wpool.tile([C2, C * 9], FP32)
        nc.sync.dma_start(w1s[:], w1[:].rearrange("o i ky kx -> o (i ky kx)"))
        w1t = wpool.tile([128, 9, C2], BF16)
        for tap in range(9):
            pt = psA.tile([C, C2], FP32, tag="ps")
            sv = w1s[:].rearrange("o (i t) -> o t i", t=9)[:, tap, :]
            nc.tensor.transpose(pt[:], sv, ident[:])
            nc.vector.tensor_copy(w1t[0:CH, tap, :], pt[:])
            nc.vector.tensor_copy(w1t[CH:128, tap, :], pt[:])

        w2s = wpool.tile([C2, C2 * 9], FP32)
        nc.sync.dma_start(w2s[:], w2[:].rearrange("o i ky kx -> o (i ky kx)"))
        w2t = wpool.tile([128, 9, C2], BF16)
        for tap in range(9):
            pt = psA.tile([C2, C2], FP32, tag="ps")
            sv = w2s[:].rearrange("o (i t) -> o t i", t=9)[:, tap, :]
            nc.tensor.transpose(pt[:], sv, ident[:])
            nc.vector.tensor_copy(w2t[:, tap, :], pt[:])

        atts = wpool.tile([C, C], FP32)
        nc.sync.dma_start(atts[:], attw[:, :, 0, 0])
        attt = wpool.tile([128, C], BF16)
        pt = psA.tile([C, C], FP32, tag="ps")
        nc.tensor.transpose(pt[:], atts[:], ident[0:C, 0:C])
        nc.vector.tensor_copy(attt[0:CH, :], pt[:])
        nc.vector.tensor_copy(attt[CH:128, :], pt[:])

        fc1t = wpool.tile([C, C // 16], FP32)
        nc.sync.dma_start(fc1t[:], fc1[:, :, 0, 0].rearrange("o c -> c o"))
        fc1b = wpool.tile([C, C // 16], BF16)
        nc.vector.tensor_copy(fc1b[:], fc1t[:])
        fc2t = wpool.tile([C // 16, C], FP32)
        nc.sync.dma_start(fc2t[:], fc2[:, :, 0, 0].rearrange("o c -> c o"))
        fc2b = wpool.tile([C // 16, C], BF16)
        nc.vector.tensor_copy(fc2b[:], fc2t[:])

        b1t = wpool.tile([C2, 1], FP32)
        nc.sync.dma_start(b1t[:], b1[:, None])
        b2t = wpool.tile([C2, 1], FP32)
        nc.sync.dma_start(b2t[:], b2[:, None])
        attbt = wpool.tile([C, 1], FP32)
        nc.sync.dma_start(attbt[:], attb[:, None])

        # ---------------- load + maxpool ----------------
        xp_tiles = [
            pxp.tile([128, XPC, w], BF16, tag=f"xp{i}", name=f"xp{i}")
            for i in range(n_dct)
        ]
        for k in range(n_mp):
            xin = pin.tile([128, 2 * MP_CH, W], FP32, tag="xin")
            r0 = 2 * MP_CH * k
            nc.sync.dma_start(xin[0:CH, :, :], x[:, r0 : r0 + 2 * MP_CH, :])
            nc.sync.dma_start(
                xin[CH:128, :, :], x[:, H // 2 + r0 : H // 2 + r0 + 2 * MP_CH, :]
            )
            hmax = pin.tile([128, 2 * MP_CH, w], BF16, tag="hmax")
            xv = xin[:].rearrange("p r (a two) -> p r a two", two=2)
            nc.vector.tensor_tensor(hmax[:], xv[:, :, :, 0], xv[:, :, :, 1], OP.max)
            xpt = xp_tiles[k // mp_per_xpc]
            rr = (k % mp_per_xpc) * MP_CH
            hv = hmax[:].rearrange("p (b two) q -> p b two q", two=2)
            nc.vector.tensor_tensor(
                xpt[:, rr : rr + MP_CH, :], hv[:, :, 0, :], hv[:, :, 1, :], OP.max
            )

        pin.release()

        # ---------------- DCT ----------------
        recon = prec.tile([128, hh, w], BF16)  # pre-gather reconstruction
        shp4 = (128, DCT_T, S, N)
        for c in range(n_dct):
            xpt = xp_tiles[c]
            a0 = smallD.tile([128, DCT_T, w], FP32, tag="a0")
            nc.vector.tensor_reduce(
                a0[:], xpt[:].rearrange("p (t r) q -> p t q r", r=N),
                axis=AX.X, op=OP.add,
            )
            a1 = smallD.tile([128, DCT_T, w], FP32, tag="a1")
            xv_row = xpt[:].rearrange("p (t r) q -> p t r q", r=N)
            for r in range(N):
                if r == 0:
                    nc.vector.tensor_scalar(
                        a1[:], xv_row[:, :, r, :], float(COS1[0]), None, OP.mult
                    )
                else:
                    nc.vector.scalar_tensor_tensor(
                        a1[:], xv_row[:, :, r, :], float(COS1[r]), a1[:],
                        OP.mult, OP.add,
                    )
            a0v = a0[:].rearrange("p t (s q) -> p t s q", q=N)
            a1v = a1[:].rearrange("p t (s q) -> p t s q", q=N)
            c00 = smallD.tile([128, DCT_T, S], FP32, tag="c00")
            nc.vector.tensor_reduce(c00[:], a0v, axis=AX.X, op=OP.add)
            c10 = smallD.tile([128, DCT_T, S], FP32, tag="c10")
            nc.vector.tensor_reduce(c10[:], a1v, axis=AX.X, op=OP.add)
            tmp = smallD.tile([128, DCT_T, S, N], FP32, tag="ctmp")
            nc.vector.tensor_tensor(tmp[:], a0v, cvec(2, shp4), OP.mult)
            c01 = smallD.tile([128, DCT_T, S], FP32, tag="c01")
            nc.vector.tensor_reduce(c01[:], tmp[:], axis=AX.X, op=OP.add)
            nc.vector.tensor_tensor(tmp[:], a0v, cvec(3, shp4), OP.mult)
            c02 = smallD.tile([128, DCT_T, S], FP32, tag="c02")
            nc.vector.tensor_reduce(c02[:], tmp[:], axis=AX.X, op=OP.add)

            e0 = smallD.tile([128, DCT_T, w], BF16, tag="e0")
            e0v = e0[:].rearrange("p t (s q) -> p t s q", q=N)
            tmp8 = smallD.tile([128, DCT_T, w], BF16, tag="tmp8")
            tmp8v = tmp8[:].rearrange("p t (s q) -> p t s q", q=N)
            c01b = c01[:, :, :, None].to_broadcast(shp4)
            c02b = c02[:, :, :, None].to_broadcast(shp4)
            c00b = c00[:, :, :, None].to_broadcast(shp4)
            nc.vector.tensor_tensor(e0v, c01b, cvec(0, shp4), OP.mult)
            nc.vector.tensor_tensor(tmp8v, c02b, cvec(1, shp4), OP.mult)
            nc.vector.tensor_tensor(e0[:], e0[:], tmp8[:], OP.add)
            nc.vector.scalar_tensor_tensor(e0v, c00b, A00, e0v, OP.mult, OP.add)

            c10e = smallD.tile([128, DCT_T, w], BF16, tag="c10e")
            c10ev = c10e[:].rearrange("p t (s q) -> p t s q", q=N)
            nc.scalar.copy(c10ev, c10[:, :, :, None].to_broadcast(shp4))

            rv = recon[:, c * XPC : (c + 1) * XPC, :].rearrange(
                "p (t r) q -> p t r q", r=N
            )
            for r in range(N):
                nc.vector.scalar_tensor_tensor(
                    rv[:, :, r, :], c10e[:], float(A10 * COS1[r]), e0[:],
                    OP.mult, OP.add,
                )

        # ---------------- gather rows+cols into y1 ----------------
        y1 = py1.tile([128, hh, w], BF16)
        for hf in (0, 1):
            pb = hf * CH
            use_act = hf == 1  # spread across both engines
            for ro, rin_g, rl in row_runs_h[hf]:
                for ro2, rin2, rl2 in _split_at(ro, rin_g, rl, hh):
                    src_hf = 0 if rin2 < hh else 1
                    rin_l = rin2 - hh * src_hf
                    pbi = src_hf * CH
                    if pbi != pb:
                        # cross-half rows: bounce through DMA into a
                        # base-aligned staging tile
                        xstage = small.tile([128, N, w], BF16, tag="xstage")
                        nc.sync.dma_start(
                            xstage[pb : pb + CH, 0:rl2, :],
                            recon[pbi : pbi + CH, rin_l : rin_l + rl2, :],
                        )
                        srct = xstage
                        srow = 0
                        spb = pb
                    else:
                        srct = recon
                        srow = rin_l
                        spb = pbi
                    for co, cin, cl in col_runs:
                        src = srct[spb : spb + CH, srow : srow + rl2, cin : cin + cl]
                        dst = y1[pb : pb + CH, ro2 : ro2 + rl2, co : co + cl]
                        if use_act:
                            nc.scalar.copy(dst, src)
                        else:
                            nc.vector.tensor_copy(dst, src)

        # sums for SE stats (scratch output into dead recon buffer)
        ysum = small.tile([128, 1], FP32, tag="ysum")
        nc.scalar.activation(recon[:], y1[:], ACT.Copy, accum_out=ysum[:])
        ysq = small.tile([128, 1], FP32, tag="ysq")
        nc.scalar.activation(recon[:], y1[:], ACT.Square, accum_out=ysq[:])

        prec.release()

        if debug:
            xpd = dpool.tile([128, hh, w], FP32, tag="xpd")
            for c in range(n_dct):
                nc.vector.tensor_copy(
                    xpd[:, c * XPC : (c + 1) * XPC, :], xp_tiles[c][:]
                )
            nc.sync.dma_start(dbg["dbg_xp"][:], xpd[:])
            y1d = dpool.tile([128, hh, w], FP32, tag="y1d")
            nc.vector.tensor_copy(y1d[:], y1[:])
            nc.sync.dma_start(dbg["dbg_y1"][:], y1d[:])

        # ---------------- SE ----------------
        st = small.tile([64, 12], FP32, tag="se")
        yhi = small.tile([64, 2], FP32, tag="yhi")
        nc.sync.dma_start(yhi[:, 0:1], ysum[CH:128, :])
        nc.sync.dma_start(yhi[:, 1:2], ysq[CH:128, :])
        nc.vector.tensor_tensor(st[:, 0:1], ysum[0:CH, :], yhi[:, 0:1], OP.add)
        nc.vector.tensor_tensor(st[:, 1:2], ysq[0:CH, :], yhi[:, 1:2], OP.add)
        nc.vector.tensor_scalar(st[:, 2:3], st[:, 0:1], 1.0 / NF, None, OP.mult)
        nc.vector.tensor_scalar(st[:, 3:4], st[:, 1:2], 1.0 / NF, None, OP.mult)
        nc.vector.tensor_tensor(st[:, 4:5], st[:, 2:3], st[:, 2:3], OP.mult)
        nc.vector.tensor_tensor(st[:, 5:6], st[:, 3:4], st[:, 4:5], OP.subtract)
        nc.vector.tensor_scalar(
            st[:, 6:7], st[:, 5:6], float(NF) / float(NF - 1), None, OP.mult
        )
        nc.vector.tensor_tensor(st[:, 7:8], st[:, 2:3], st[:, 6:7], OP.add)
        sb = small.tile([64, 1], BF16, tag="sb16")
        nc.vector.tensor_copy(sb[:], st[:, 7:8])
        pfc1 = psA.tile([C // 16, 1], FP32, tag="ps")
        nc.tensor.matmul(pfc1[:], fc1b[:], sb[:], start=True, stop=True)
        tb = small.tile([C // 16, 1], BF16, tag="tb16")
        nc.scalar.activation(tb[:], pfc1[:], ACT.Relu)
        pfc2 = psA.tile([C, 1], FP32, tag="ps")
        nc.tensor.matmul(pfc2[:], fc2b[:], tb[:], start=True, stop=True)
        gamma = small.tile([64, 1], FP32, tag="gamma")
        nc.scalar.activation(gamma[:], pfc2[:], ACT.Sigmoid)
        gamma128 = small.tile([128, 1], FP32, tag="g128")
        nc.vector.tensor_copy(gamma128[0:CH, :], gamma[:])
        nc.sync.dma_start(gamma128[CH:128, :], gamma[:])
        if debug:
            nc.sync.dma_start(dbg["dbg_se"][:], st[:, 0:8])
            nc.sync.dma_start(dbg["dbg_gamma"][:], gamma[:])

        # ---------------- x_all = xp - y1, then y1 *= gamma (in place) ----------------
        pxa = tc.alloc_tile_pool(name="pxa", bufs=1)
        x_all = pxa.tile([128, hh + 2, w + 2], BF16)
        nc.vector.memset(x_all[:, :, 0], 0.0)
        nc.vector.memset(x_all[:, :, w + 1], 0.0)
        nc.vector.memset(x_all[0:CH, 0, :], 0.0)
        nc.vector.memset(x_all[CH:128, hh + 1, :], 0.0)

        for c in range(n_dct):
            nc.vector.tensor_tensor(
                x_all[:, 1 + c * XPC : 1 + (c + 1) * XPC, 1 : w + 1],
                xp_tiles[c][:],
                y1[:, c * XPC : (c + 1) * XPC, :],
                OP.subtract,
            )
        pxp.release()
        y1g = y1
        nc.vector.tensor_scalar(y1g[:], y1[:], gamma128[:, 0:1], None, OP.mult)

        ATT_G = 16 if hh % 16 == 0 else 8
        n_att_g = hh // ATT_G
        FLAT = ATT_G * w
        AN = 512 if FLAT % 512 == 0 else 384
        n_fl = FLAT // AN
        for hf in (0, 1):
            pb = hf * CH
            for g in range(n_att_g):
                xc = small.tile([128, ATT_G, w], BF16, tag="xc")
                base = g * FLAT
                y1gv = y1g[pb : pb + CH, :, :].rearrange("p a b -> p (a b)")
                xcv = xc[pb : pb + CH, :, :].rearrange("p a b -> p (a b)")
                for f in range(n_fl):
                    pa = psA.tile([C, AN], FP32, tag="ps")
                    nc.tensor.matmul(
                        pa[:],
                        attt[pb : pb + CH, :],
                        y1gv[:, base + f * AN : base + (f + 1) * AN],
                        start=True,
                        stop=True,
                    )
                    nc.scalar.activation(
                        xcv[:, f * AN : (f + 1) * AN], pa[:], ACT.Relu,
                        bias=attbt[:, 0:1],
                    )
                sl = x_all[pb : pb + CH, 1 + g * ATT_G : 1 + (g + 1) * ATT_G, 1 : w + 1]
                nc.vector.tensor_tensor(sl, sl, xc[pb : pb + CH, :, :], OP.add)

        nc.sync.dma_start(x_all[CH:128, 0, :], x_all[0:CH, hh, :])
        nc.sync.dma_start(x_all[0:CH, hh + 1, :], x_all[CH:128, 1, :])
        if debug:
            xad = dpool.tile([128, hh + 2, w + 2], FP32, tag="xad")
            nc.vector.tensor_copy(xad[:], x_all[:])
            nc.sync.dma_start(dbg["dbg_xall"][:], xad[:])

        # ---------------- conv1 -> o1 ----------------
        po1 = tc.alloc_tile_pool(name="po1", bufs=1)
        o1 = po1.tile([C2, h + 2, w + 2], BF16)
        nc.vector.memset(o1[:, 0, :], 0.0)
        nc.vector.memset(o1[:, h + 1, :], 0.0)
        nc.vector.memset(o1[:, :, 0], 0.0)
        nc.vector.memset(o1[:, :, w + 1], 0.0)

        RT = 2
        n_c1 = hh // RT
        for hf in (0, 1):
            pb = hf * CH
            for g in range(n_c1):
                pc = psC.tile([C2, RT * w], FP32, tag="pc")
                lr = g * RT
                for tap in range(9):
                    dy, dx = divmod(tap, 3)
                    rhs = x_all[pb : pb + CH, lr + dy : lr + dy + RT, dx : dx + w]
                    nc.tensor.matmul(
                        pc[:], w1t[pb : pb + CH, tap, :], rhs,
                        start=(tap == 0), stop=(tap == 8),
                    )
                grow = hf * hh + lr
                dst = o1[:, 1 + grow : 1 + grow + RT, 1 : w + 1]
                if g % 2 == 0:
                    nc.scalar.activation(dst, pc[:], ACT.Relu, bias=b1t[:, 0:1])
                else:
                    nc.vector.scalar_tensor_tensor(
                        dst, pc[:], b1t[:, 0:1],
                        zerot[:, 0:1, None].to_broadcast((C2, RT, w)),
                        OP.add, OP.max,
                    )
        if debug:
            o1d = dpool.tile([C2, h + 2, w + 2], FP32, tag="o1d")
            nc.vector.tensor_copy(o1d[:], o1[:])
            nc.sync.dma_start(dbg["dbg_o1"][:], o1d[:])

        # ---------------- conv2 -> out ----------------
        n_c2 = h // RT
        for g in range(n_c2):
            pc = psC.tile([C2, RT * w], FP32, tag="pc")
            lr = g * RT
            for tap in range(9):
                dy, dx = divmod(tap, 3)
                rhs = o1[:, lr + dy : lr + dy + RT, dx : dx + w]
                nc.tensor.matmul(
                    pc[:], w2t[:, tap, :], rhs, start=(tap == 0), stop=(tap == 8)
                )
            stg = small.tile([C2, RT * w], FP32, tag="ostg")
            if g % 2 == 0:
                nc.scalar.activation(stg[:], pc[:], ACT.Relu, bias=b2t[:, 0:1])
            else:
                nc.vector.scalar_tensor_tensor(
                    stg[:], pc[:], b2t[:, 0:1],
                    zerot[:, 0:1].to_broadcast((C2, RT * w)),
                    OP.add, OP.max,
                )
            nc.sync.dma_start(out[:, lr : lr + RT, :], stg[:])

        po1.release()
        pxa.release()
        py1.release()

    nc.finalize()
    return nc


_NC_CACHE = {}


def _get_nc(H=384, W=384, debug=False):
    key = (H, W, debug)
    if key not in _NC_CACHE:
        _NC_CACHE[key] = build_nc(H=H, W=W, debug=debug)
    return _NC_CACHE[key]


def kernel(x, w1, b1, w2, b2, att_conv_w, att_conv_b, fc1_w, fc2_w):
    x = np.ascontiguousarray(np.asarray(x, np.float32))
    B = x.shape[0]
    nc = _get_nc(x.shape[2], x.shape[3])
    shared = {
        "w1": np.ascontiguousarray(np.asarray(w1, np.float32)),
        "b1": np.ascontiguousarray(np.asarray(b1, np.float32)),
        "w2": np.ascontiguousarray(np.asarray(w2, np.float32)),
        "b2": np.ascontiguousarray(np.asarray(b2, np.float32)),
        "att_conv_w": np.ascontiguousarray(np.asarray(att_conv_w, np.float32)),
        "att_conv_b": np.ascontiguousarray(np.asarray(att_conv_b, np.float32)),
        "fc1_w": np.ascontiguousarray(np.asarray(fc1_w, np.float32)),
        "fc2_w": np.ascontiguousarray(np.asarray(fc2_w, np.float32)),
    }
    in_maps = [dict(shared, x=np.ascontiguousarray(x[i])) for i in range(B)]
    res = run_bass_kernel_spmd(nc, in_maps, core_ids=list(range(B)))
    return np.stack([res.results[i]["out"] for i in range(B)], axis=0)



# revision 2
# speedup vs baseline: 2.1531x; 2.1531x over previous
"""Trainium2 Bass kernel for nn_DownBlock_res_dct1 (maxpool 2x2 + truncated
block-DCT low-pass + SE attention + 1x1 conv + two 3x3 convs), data-parallel
over the batch across 8 NeuronCores.

Self-contained: hardcodes all shapes/constants; builds one SPMD Bass module
(one batch item per core), runs via run_bass_kernel_spmd, gathers the full
(8, 128, 192, 192) output.

Per-core layout: partitions p = half*64 + ch, where half selects image row
halves. All pre-conv compute runs in this layout; convs run per-half with
K=64 (conv1/att) or K=128 (conv2) contractions, N = 2 image rows per PSUM
tile. Matmuls in bf16 with fp32 PSUM accumulation.
"""

import math
from contextlib import ExitStack

import numpy as np

import concourse.bass as bass
import concourse.mybir as mybir
import concourse.tile as tile
from concourse import bacc
from concourse.bass_utils import run_bass_kernel_spmd

FP32 = mybir.dt.float32
BF16 = mybir.dt.bfloat16
AX = mybir.AxisListType
OP = mybir.AluOpType
ACT = mybir.ActivationFunctionType

N = 8  # DCT block size
_P8 = np.arange(8)
COS1 = np.cos(math.pi * (_P8 + 0.5) / 8.0 * 1).astype(np.float64)
COS2 = np.cos(math.pi * (_P8 + 0.5) / 8.0 * 2).astype(np.float64)
# Selected zigzag coeffs [0,1,2,5] -> (k1,k2) in {(0,0),(0,1),(1,0),(0,2)}
A00 = (1.0 / 8.0) ** 2
A01 = 2.0 / 64.0
A02 = 2.0 / 64.0
A10 = 2.0 / 64.0


def _runs(idx):
    """Contiguous runs where idx[i] = i - g: list of (out_start, in_start, len)."""
    runs = []
    s = 0
    for i in range(1, len(idx) + 1):
        if i == len(idx) or idx[i] != idx[i - 1] + 1:
            runs.append((s, int(idx[s]), i - s))
            s = i
    return runs


def _split_at(ro, rin, rl, bound):
    """Split a run at source-row `bound`."""
    if rin < bound < rin + rl:
        return [(ro, rin, bound - rin), (ro + bound - rin, bound, rin + rl - bound)]
    return [(ro, rin, rl)]


def build_nc(H=384, W=384, debug=False):
    C, C2 = 64, 128
    h, w = H // 2, W // 2
    hh = h // 2  # rows per half
    assert hh % N == 0 and w % N == 0
    T = hh // N  # block-rows per half
    S = w // N  # block-cols
    CH = 64

    hi = (np.arange(h) * (h - (N - 1))) // h
    wi = (np.arange(w) * (w - (N - 1))) // w
    col_runs = _runs(wi)
    row_runs_h = [_runs(hi[hh * hf : hh * (hf + 1)]) for hf in (0, 1)]

    MP_CH = 4  # pooled rows per maxpool chunk
    n_mp = hh // MP_CH
    DCT_T = 2 if T % 2 == 0 else T
    n_dct = T // DCT_T
    XPC = DCT_T * N  # xp tile rows
    mp_per_xpc = XPC // MP_CH
    assert XPC % MP_CH == 0

    nc = bacc.Bacc("TRN2")

    x = nc.dram_tensor("x", [C, H, W], FP32, kind="ExternalInput")
    w1 = nc.dram_tensor("w1", [C2, C, 3, 3], FP32, kind="ExternalInput")
    b1 = nc.dram_tensor("b1", [C2], FP32, kind="ExternalInput")
    w2 = nc.dram_tensor("w2", [C2, C2, 3, 3], FP32, kind="ExternalInput")
    b2 = nc.dram_tensor("b2", [C2], FP32, kind="ExternalInput")
    attw = nc.dram_tensor("att_conv_w", [C, C, 1, 1], FP32, kind="ExternalInput")
    attb = nc.dram_tensor("att_conv_b", [C], FP32, kind="ExternalInput")
    fc1 = nc.dram_tensor("fc1_w", [C // 16, C, 1, 1], FP32, kind="ExternalInput")
    fc2 = nc.dram_tensor("fc2_w", [C, C // 16, 1, 1], FP32, kind="ExternalInput")
    out = nc.dram_tensor("out", [C2, h, w], FP32, kind="ExternalOutput")

    dbg = {}
    if debug:
        for name, shape in [
            ("dbg_xp", [128, hh, w]),
            ("dbg_y1", [128, hh, w]),
            ("dbg_se", [64, 8]),
            ("dbg_gamma", [64, 1]),
            ("dbg_xall", [128, hh + 2, w + 2]),
            ("dbg_o1", [C2, h + 2, w + 2]),
        ]:
            dbg[name] = nc.dram_tensor(name, shape, FP32, kind="ExternalOutput")

    const_np = np.zeros((128, 4, 8), np.float32)
    const_np[:, 0, :] = COS1
    const_np[:, 1, :] = COS2
    const_np[:, 2, :] = COS1 * A01
    const_np[:, 3, :] = COS2 * A02
    cdram = nc.inline_tensor(const_np.reshape(128, 32), name="dctconst")

    NF = h * w  # pixels per full channel image

    with tile.TileContext(nc) as tc, ExitStack() as ctx:
        wpool = ctx.enter_context(tc.tile_pool(name="wpool", bufs=1))
        if debug:
            dpool = ctx.enter_context(tc.tile_pool(name="dpool", bufs=1))
        smallD = ctx.enter_context(tc.tile_pool(name="smallD", bufs=1))
        small = ctx.enter_context(tc.tile_pool(name="small", bufs=2))
        psA = ctx.enter_context(tc.tile_pool(name="psA", bufs=2, space="PSUM"))
        psC = ctx.enter_context(tc.tile_pool(name="psC", bufs=4, space="PSUM"))
        # phase-scoped pools; each SBUF side is a LIFO stack.
        # left:  ... pxp prec | pop prec, pop pxp, push pxa, push po1
        # right: py1 pin      | pop pin (after maxpool), pop py1 (at end)
        py1 = tc.alloc_tile_pool(name="py1", bufs=1, side="right")
        pxp = tc.alloc_tile_pool(name="pxp", bufs=1, side="right")
        pin = tc.alloc_tile_pool(name="pin", bufs=2, side="right")
        prec = tc.alloc_tile_pool(name="prec", bufs=1)

        # ---------------- constants / weights ----------------
        consts = wpool.tile([128, 4, 8], FP32)
        nc.sync.dma_start(consts[:], cdram[:].rearrange("p (a b) -> p a b", a=4))

        def cvec(row, shp):  # broadcast [128,8] const row to shp (q innermost)
            return consts[:, row, None, None, :].to_broadcast(shp)

        from concourse.masks import make_identity

        ident = wpool.tile([128, 128], FP32)
        make_identity(nc, ident[:])

        zerot = wpool.tile([128, 1], FP32)
        nc.vector.memset(zerot[:], 0.0)

        w1s = wpool.tile([C2, C * 9], FP32)
        nc.sync.dma_start(w1s[:], w1[:].rearrange("o i ky kx -> o (i ky kx)"))
        w1t = wpool.tile([128, 9, C2], BF16)
        for tap in range(9):
            pt = psA.tile([C, C2], FP32, tag="ps")
            sv = w1s[:].rearrange("o (i t) -> o t i", t=9)[:, tap, :]
            nc.tensor.transpose(pt[:], sv, ident[:])
            nc.vector.tensor_copy(w1t[0:CH, tap, :], pt[:])
            nc.vector.tensor_copy(w1t[CH:128, tap, :], pt[:])

        w2s = wpool.tile([C2, C2 * 9], FP32)
        nc.sync.dma_start(w2s[:], w2[:].rearrange("o i ky kx -> o (i ky kx)"))
        w2t = wpool.tile([128, 9, C2], BF16)
        for tap in range(9):
            pt = psA.tile([C2, C2], FP32, tag="ps")
            sv = w2s[:].rearrange("o (i t) -> o t i", t=9)[:, tap, :]
            nc.tensor.transpose(pt[:], sv, ident[:])
            nc.vector.tensor_copy(w2t[:, tap, :], pt[:])

        atts = wpool.tile([C, C], FP32)
        nc.sync.dma_start(atts[:], attw[:, :, 0, 0])
        attt = wpool.tile([128, C], BF16)
        pt = psA.tile([C, C], FP32, tag="ps")
        nc.tensor.transpose(pt[:], atts[:], ident[0:C, 0:C])
        nc.vector.tensor_copy(attt[0:CH, :], pt[:])
        nc.vector.tensor_copy(attt[CH:128, :], pt[:])

        fc1t = wpool.tile([C, C // 16], FP32)
        nc.sync.dma_start(fc1t[:], fc1[:, :, 0, 0].rearrange("o c -> c o"))
        fc1b = wpool.tile([C, C // 16], BF16)
        nc.vector.tensor_copy(fc1b[:], fc1t[:])
        fc2t = wpool.tile([C // 16, C], FP32)
        nc.sync.dma_start(fc2t[:], fc2[:, :, 0, 0].rearrange("o c -> c o"))
        fc2b = wpool.tile([C // 16, C], BF16)
        nc.vector.tensor_copy(fc2b[:], fc2t[:])

        b1t = wpool.tile([C2, 1], FP32)
        nc.sync.dma_start(b1t[:], b1[:, None])
        b2t = wpool.tile([C2, 1], FP32)
        nc.sync.dma_start(b2t[:], b2[:, None])
        attbt = wpool.tile([C, 1], FP32)
        nc.sync.dma_start(attbt[:], attb[:, None])

        # ---------------- load + maxpool ----------------
        xp_tiles = [
            pxp.tile([128, XPC, w], BF16, tag=f"xp{i}", name=f"xp{i}")
            for i in range(n_dct)
        ]
        for k in range(n_mp):
            xin = pin.tile([128, 2 * MP_CH, W], FP32, tag="xin")
            r0 = 2 * MP_CH * k
            nc.sync.dma_start(xin[0:CH, :, :], x[:, r0 : r0 + 2 * MP_CH, :])
            nc.sync.dma_start(
                xin[CH:128, :, :], x[:, H // 2 + r0 : H // 2 + r0 + 2 * MP_CH, :]
            )
            hmax = pin.tile([128, 2 * MP_CH, w], BF16, tag="hmax")
            xv = xin[:].rearrange("p r (a two) -> p r a two", two=2)
            nc.vector.tensor_tensor(hmax[:], xv[:, :, :, 0], xv[:, :, :, 1], OP.max)
            xpt = xp_tiles[k // mp_per_xpc]
            rr = (k % mp_per_xpc) * MP_CH
            hv = hmax[:].rearrange("p (b two) q -> p b two q", two=2)
            nc.vector.tensor_tensor(
                xpt[:, rr : rr + MP_CH, :], hv[:, :, 0, :], hv[:, :, 1, :], OP.max
            )

        pin.release()

        # ---------------- DCT ----------------
        recon = prec.tile([128, hh, w], BF16)  # pre-gather reconstruction
        shp4 = (128, DCT_T, S, N)
        for c in range(n_dct):
            xpt = xp_tiles[c]
            a0 = smallD.tile([128, DCT_T, w], FP32, tag="a0")
            nc.vector.tensor_reduce(
                a0[:], xpt[:].rearrange("p (t r) q -> p t q r", r=N),
                axis=AX.X, op=OP.add,
            )
            a1 = smallD.tile([128, DCT_T, w], FP32, tag="a1")
            xv_row = xpt[:].rearrange("p (t r) q -> p t r q", r=N)
            for r in range(N):
                if r == 0:
                    nc.vector.tensor_scalar(
                        a1[:], xv_row[:, :, r, :], float(COS1[0]), None, OP.mult
                    )
                else:
                    nc.vector.scalar_tensor_tensor(
                        a1[:], xv_row[:, :, r, :], float(COS1[r]), a1[:],
                        OP.mult, OP.add,
                    )
            a0v = a0[:].rearrange("p t (s q) -> p t s q", q=N)
            a1v = a1[:].rearrange("p t (s q) -> p t s q", q=N)
            c00 = smallD.tile([128, DCT_T, S], FP32, tag="c00")
            nc.vector.tensor_reduce(c00[:], a0v, axis=AX.X, op=OP.add)
            c10 = smallD.tile([128, DCT_T, S], FP32, tag="c10")
            nc.vector.tensor_reduce(c10[:], a1v, axis=AX.X, op=OP.add)
            tmp = smallD.tile([128, DCT_T, S, N], FP32, tag="ctmp")
            nc.vector.tensor_tensor(tmp[:], a0v, cvec(2, shp4), OP.mult)
            c01 = smallD.tile([128, DCT_T, S], FP32, tag="c01")
            nc.vector.tensor_reduce(c01[:], tmp[:], axis=AX.X, op=OP.add)
            nc.vector.tensor_tensor(tmp[:], a0v, cvec(3, shp4), OP.mult)
            c02 = smallD.tile([128, DCT_T, S], FP32, tag="c02")
            nc.vector.tensor_reduce(c02[:], tmp[:], axis=AX.X, op=OP.add)

            e0 = smallD.tile([128, DCT_T, w], BF16, tag="e0")
            e0v = e0[:].rearrange("p t (s q) -> p t s q", q=N)
            tmp8 = smallD.tile([128, DCT_T, w], BF16, tag="tmp8")
            tmp8v = tmp8[:].rearrange("p t (s q) -> p t s q", q=N)
            c01b = c01[:, :, :, None].to_broadcast(shp4)
            c02b = c02[:, :, :, None].to_broadcast(shp4)
            c00b = c00[:, :, :, None].to_broadcast(shp4)
            nc.vector.tensor_tensor(e0v, c01b, cvec(0, shp4), OP.mult)
            nc.vector.tensor_tensor(tmp8v, c02b, cvec(1, shp4), OP.mult)
            nc.vector.tensor_tensor(e0[:], e0[:], tmp8[:], OP.add)
            nc.vector.scalar_tensor_tensor(e0v, c00b, A00, e0v, OP.mult, OP.add)

            c10e = smallD.tile([128, DCT_T, w], BF16, tag="c10e")
            c10ev = c10e[:].rearrange("p t (s q) -> p t s q", q=N)
            nc.scalar.copy(c10ev, c10[:, :, :, None].to_broadcast(shp4))

            rv = recon[:, c * XPC : (c + 1) * XPC, :].rearrange(
                "p (t r) q -> p t r q", r=N
            )
            for r in range(N):
                nc.vector.scalar_tensor_tensor(
                    rv[:, :, r, :], c10e[:], float(A10 * COS1[r]), e0[:],
                    OP.mult, OP.add,
                )

        # ---------------- gather rows+cols into y1 ----------------
        y1 = py1.tile([128, hh, w], BF16)
        for hf in (0, 1):
            pb = hf * CH
            use_act = hf == 1  # spread across both engines
            for ro, rin_g, rl in row_runs_h[hf]:
                for ro2, rin2, rl2 in _split_at(ro, rin_g, rl, hh):
                    src_hf = 0 if rin2 < hh else 1
                    rin_l = rin2 - hh * src_hf
                    pbi = src_hf * CH
                    if pbi != pb:
                        # cross-half rows: bounce through DMA into a
                        # base-aligned staging tile
                        xstage = small.tile([128, N, w], BF16, tag="xstage")
                        nc.sync.dma_start(
                            xstage[pb : pb + CH, 0:rl2, :],
                            recon[pbi : pbi + CH, rin_l : rin_l + rl2, :],
                        )
                        srct = xstage
                        srow = 0
                        spb = pb
                    else:
                        srct = recon
                        srow = rin_l
                        spb = pbi
                    for co, cin, cl in col_runs:
                        src = srct[spb : spb + CH, srow : srow + rl2, cin : cin + cl]
                        dst = y1[pb : pb + CH, ro2 : ro2 + rl2, co : co + cl]
                        if use_act:
                            nc.scalar.copy(dst, src)
                        else:
                            nc.vector.tensor_copy(dst, src)

        # sums for SE stats (scratch output into dead recon buffer)
        ysum = small.tile([128, 1], FP32, tag="ysum")
        nc.scalar.activation(recon[:], y1[:], ACT.Copy, accum_out=ysum[:])
        ysq = small.tile([128, 1], FP32, tag="ysq")
        nc.scalar.activation(recon[:], y1[:], ACT.Square, accum_out=ysq[:])

        prec.release()

        if debug:
            xpd = dpool.tile([128, hh, w], FP32, tag="xpd")
            for c in range(n_dct):
                nc.vector.tensor_copy(
                    xpd[:, c * XPC : (c + 1) * XPC, :], xp_tiles[c][:]
                )
            nc.sync.dma_start(dbg["dbg_xp"][:], xpd[:])
            y1d = dpool.tile([128, hh, w], FP32, tag="y1d")
            nc.vector.tensor_copy(y1d[:], y1[:])
            nc.sync.dma_start(dbg["dbg_y1"][:], y1d[:])

        # ---------------- SE ----------------
        st = small.tile([64, 12], FP32, tag="se")
        yhi = small.tile([64, 2], FP32, tag="yhi")
        nc.sync.dma_start(yhi[:, 0:1], ysum[CH:128, :])
        nc.sync.dma_start(yhi[:, 1:2], ysq[CH:128, :])
        nc.vector.tensor_tensor(st[:, 0:1], ysum[0:CH, :], yhi[:, 0:1], OP.add)
        nc.vector.tensor_tensor(st[:, 1:2], ysq[0:CH, :], yhi[:, 1:2], OP.add)
        nc.vector.tensor_scalar(st[:, 2:3], st[:, 0:1], 1.0 / NF, None, OP.mult)
        nc.vector.tensor_scalar(st[:, 3:4], st[:, 1:2], 1.0 / NF, None, OP.mult)
        nc.vector.tensor_tensor(st[:, 4:5], st[:, 2:3], st[:, 2:3], OP.mult)
        nc.vector.tensor_tensor(st[:, 5:6], st[:, 3:4], st[:, 4:5], OP.subtract)
        nc.vector.tensor_scalar(
            st[:, 6:7], st[:, 5:6], float(NF) / float(NF - 1), None, OP.mult
        )
        nc.vector.tensor_tensor(st[:, 7:8], st[:, 2:3], st[:, 6:7], OP.add)
        sb = small.tile([64, 1], BF16, tag="sb16")
        nc.vector.tensor_copy(sb[:], st[:, 7:8])
        pfc1 = psA.tile([C // 16, 1], FP32, tag="ps")
        nc.tensor.matmul(pfc1[:], fc1b[:], sb[:], start=True, stop=True)
        tb = small.tile([C // 16, 1], BF16, tag="tb16")
        nc.scalar.activation(tb[:], pfc1[:], ACT.Relu)
        pfc2 = psA.tile([C, 1], FP32, tag="ps")
        nc.tensor.matmul(pfc2[:], fc2b[:], tb[:], start=True, stop=True)
        gamma = small.tile([64, 1], FP32, tag="gamma")
        nc.scalar.activation(gamma[:], pfc2[:], ACT.Sigmoid)
        gamma128 = small.tile([128, 1], FP32, tag="g128")
        nc.vector.tensor_copy(gamma128[0:CH, :], gamma[:])
        nc.sync.dma_start(gamma128[CH:128, :], gamma[:])
        if debug:
            nc.sync.dma_start(dbg["dbg_se"][:], st[:, 0:8])
            nc.sync.dma_start(dbg["dbg_gamma"][:], gamma[:])

        # ---------------- x_all = xp - y1, then y1 *= gamma (in place) ----------------
        pxa = tc.alloc_tile_pool(name="pxa", bufs=1)
        x_all = pxa.tile([128, hh + 2, w + 2], BF16)
        nc.vector.memset(x_all[:, :, 0], 0.0)
        nc.vector.memset(x_all[:, :, w + 1], 0.0)
        nc.vector.memset(x_all[0:CH, 0, :], 0.0)
        nc.vector.memset(x_all[CH:128, hh + 1, :], 0.0)

        for c in range(n_dct):
            nc.vector.tensor_tensor(
                x_all[:, 1 + c * XPC : 1 + (c + 1) * XPC, 1 : w + 1],
                xp_tiles[c][:],
                y1[:, c * XPC : (c + 1) * XPC, :],
                OP.subtract,
            )
        pxp.release()
        y1g = y1
        nc.vector.tensor_scalar(y1g[:], y1[:], gamma128[:, 0:1], None, OP.mult)

        ATT_G = 16 if hh % 16 == 0 else 8
        n_att_g = hh // ATT_G
        FLAT = ATT_G * w
        AN = 512 if FLAT % 512 == 0 else 384
        n_fl = FLAT // AN
        for hf in (0, 1):
            pb = hf * CH
            for g in range(n_att_g):
                xc = small.tile([128, ATT_G, w], BF16, tag="xc")
                base = g * FLAT
                y1gv = y1g[pb : pb + CH, :, :].rearrange("p a b -> p (a b)")
                xcv = xc[pb : pb + CH, :, :].rearrange("p a b -> p (a b)")
                for f in range(n_fl):
                    pa = psA.tile([C, AN], FP32, tag="ps")
                    nc.tensor.matmul(
                        pa[:],
                        attt[pb : pb + CH, :],
                        y1gv[:, base + f * AN : base + (f + 1) * AN],
                        start=True,
                        stop=True,
                    )
                    nc.scalar.activation(
                        xcv[:, f * AN : (f + 1) * AN], pa[:], ACT.Relu,
                        bias=attbt[:, 0:1],
                    )
                sl = x_all[pb : pb + CH, 1 + g * ATT_G : 1 + (g + 1) * ATT_G, 1 : w + 1]
                nc.vector.tensor_tensor(sl, sl, xc[pb : pb + CH, :, :], OP.add)

        nc.sync.dma_start(x_all[CH:128, 0, :], x_all[0:CH, hh, :])
        nc.sync.dma_start(x_all[0:CH, hh + 1, :], x_all[CH:128, 1, :])
        if debug:
            xad = dpool.tile([128, hh + 2, w + 2], FP32, tag="xad")
            nc.vector.tensor_copy(xad[:], x_all[:])
            nc.sync.dma_start(dbg["dbg_xall"][:], xad[:])

        # ---------------- conv1 -> o1 ----------------
        po1 = tc.alloc_tile_pool(name="po1", bufs=1)
        o1 = po1.tile([C2, h + 2, w + 2], BF16)
        nc.vector.memset(o1[:, 0, :], 0.0)
        nc.vector.memset(o1[:, h + 1, :], 0.0)
        nc.vector.memset(o1[:, :, 0], 0.0)
        nc.vector.memset(o1[:, :, w + 1], 0.0)

        RT = 2
        n_c1 = hh // RT
        for hf in (0, 1):
            pb = hf * CH
            for g in range(n_c1):
                pc = psC.tile([C2, RT * w], FP32, tag="pc")
                lr = g * RT
                for tap in range(9):
                    dy, dx = divmod(tap, 3)
                    rhs = x_all[pb : pb + CH, lr + dy : lr + dy + RT, dx : dx + w]
                    nc.tensor.matmul(
                        pc[:], w1t[pb : pb + CH, tap, :], rhs,
                        start=(tap == 0), stop=(tap == 8),
                    )
                grow = hf * hh + lr
                dst = o1[:, 1 + grow : 1 + grow + RT, 1 : w + 1]
                if g % 2 == 0:
                    nc.scalar.activation(dst, pc[:], ACT.Relu, bias=b1t[:, 0:1])
                else:
                    nc.vector.scalar_tensor_tensor(
                        dst, pc[:], b1t[:, 0:1],
                        zerot[:, 0:1, None].to_broadcast((C2, RT, w)),
                        OP.add, OP.max,
                    )
        if debug:
            o1d = dpool.tile([C2, h + 2, w + 2], FP32, tag="o1d")
            nc.vector.tensor_copy(o1d[:], o1[:])
            nc.sync.dma_start(dbg["dbg_o1"][:], o1d[:])

        # ---------------- conv2 -> out ----------------
        n_c2 = h // RT
        for g in range(n_c2):
            pc = psC.tile([C2, RT * w], FP32, tag="pc")
            lr = g * RT
            for tap in range(9):
                dy, dx = divmod(tap, 3)
                rhs = o1[:, lr + dy : lr + dy + RT, dx : dx + w]
                nc.tensor.matmul(
                    pc[:], w2t[:, tap, :], rhs, start=(tap == 0), stop=(tap == 8)
                )
            stg = small.tile([C2, RT * w], FP32, tag="ostg")
            if g % 2 == 0:
                nc.scalar.activation(stg[:], pc[:], ACT.Relu, bias=b2t[:, 0:1])
            else:
                nc.vector.scalar_tensor_tensor(
                    stg[:], pc[:], b2t[:, 0:1],
                    zerot[:, 0:1].to_broadcast((C2, RT * w)),
                    OP.add, OP.max,
                )
            nc.sync.dma_start(out[:, lr : lr + RT, :], stg[:])

        po1.release()
        pxa.release()
        py1.release()

    nc.finalize()
    return nc


_NC_CACHE = {}


def _get_nc(H=384, W=384, debug=False):
    key = (H, W, debug)
    if key not in _NC_CACHE:
        _NC_CACHE[key] = build_nc(H=H, W=W, debug=debug)
    return _NC_CACHE[key]


def kernel(x, w1, b1, w2, b2, att_conv_w, att_conv_b, fc1_w, fc2_w):
    x = np.ascontiguousarray(np.asarray(x, np.float32))
    B = x.shape[0]
    nc = _get_nc(x.shape[2], x.shape[3])
    shared = {
        "w1": np.ascontiguousarray(np.asarray(w1, np.float32)),
        "b1": np.ascontiguousarray(np.asarray(b1, np.float32)),
        "w2": np.ascontiguousarray(np.asarray(w2, np.float32)),
        "b2": np.ascontiguousarray(np.asarray(b2, np.float32)),
        "att_conv_w": np.ascontiguousarray(np.asarray(att_conv_w, np.float32)),
        "att_conv_b": np.ascontiguousarray(np.asarray(att_conv_b, np.float32)),
        "fc1_w": np.ascontiguousarray(np.asarray(fc1_w, np.float32)),
        "fc2_w": np.ascontiguousarray(np.asarray(fc2_w, np.float32)),
    }
    in_maps = [dict(shared, x=np.ascontiguousarray(x[i])) for i in range(B)]
    res = run_bass_kernel_spmd(nc, in_maps, core_ids=list(range(B)))
    return np.stack([res.results[i]["out"] for i in range(B)], axis=0)

